# revision 4
# baseline (speedup 1.0000x reference)
"""Trainium2 Bass kernel: density-ratio estimator loss.

Math (from the reference):
    csum = sum_b c[b, l, :]                  # (L, C)
    v[l, :] = trans[l] @ csum[l]             # (L, Z)
    r[b, l] = z[b, l, :] . v[l, :]           # (B, L)
    out = exp(r)

Sharding across 8 NeuronCores (full inputs in, full output out):
    - c     : sharded along L (16 steps per core). Each core reads c[:, l_slice, :]
              (all batches) so its csum slice is complete locally -> no AllReduce.
    - trans : sharded along L, host-pre-transposed to [l, c, z] so PE matmuls can
              produce v rows (l, z) directly.
    - v     : one tiny AllGather (16x256 f32 per rank -> 128x256).
    - z     : sharded along batch (256 rows per core); out shard is r^T (L, B/8).
"""

import numpy as np

B, L, ZD, CD = 2048, 128, 256, 256
NCORES = 8
BP = B // NCORES  # 256 batches per core
LP = L // NCORES  # 16 steps per core
P = 128  # SBUF partitions

NB = 8  # batches per z tile
NZT = BP // NB  # 32 z tiles
NCT = B // P  # 16 c tiles

_PROGRAM = None


def _build_program():
    import concourse.bacc as bacc
    import concourse.mybir as mybir
    import concourse.tile as tile

    f32 = mybir.dt.float32
    nc = bacc.Bacc("TRN2", target_bir_lowering=False, debug=False,
                   num_devices=NCORES)

    z_p = nc.dram_tensor("z", [BP, L, ZD], f32, kind="ExternalInput").ap()
    c_p = nc.dram_tensor("c", [B, LP, CD], f32, kind="ExternalInput").ap()
    tt_p = nc.dram_tensor("tt", [LP, CD, ZD], f32, kind="ExternalInput").ap()
    out_p = nc.dram_tensor("out", [L, BP], f32, kind="ExternalOutput").ap()

    with tile.TileContext(nc) as tc:
        with (
            tc.tile_pool(name="cpool", bufs=3) as cpool,
            tc.tile_pool(name="zpool", bufs=8) as zpool,
            tc.tile_pool(name="misc", bufs=1) as misc,
            tc.tile_pool(name="psum", bufs=1, space="PSUM") as psum,
            tc.tile_pool(name="dram", bufs=1, space="DRAM") as dram,
        ):
            ones_sb = misc.tile([P, 1], f32)
            nc.gpsimd.memset(ones_sb[:], 1.0)

            # ---- phase C: accumulate c over batch chunks -------------------
            acc = misc.tile([P, LP * CD], f32)
            c_re = c_p.rearrange("(n p) l k -> n p (l k)", p=P)
            for i in range(NCT):
                ct = cpool.tile([P, LP * CD], f32, tag="c")
                nc.sync.dma_start(ct[:], c_re[i])
                if i == 0:
                    nc.vector.tensor_copy(acc[:], ct[:])
                else:
                    nc.vector.tensor_add(acc[:], acc[:], ct[:])

            # transT: whole shard resident, partitions = c (within half)
            tt_sb = misc.tile([P, LP, 2, ZD], f32)
            nc.sync.dma_start(tt_sb[:], tt_p.rearrange("l (h p) z -> p l h z", h=2))

            # ---- csum columns: reduce partitions (batch) via ones-matmul ---
            # pc[:, 2l+h][m] = csum[l, h*128+m]
            pc = psum.tile([P, LP * 2], f32, tag="ps")
            for l in range(LP):
                for h in range(2):
                    j = 2 * l + h
                    nc.tensor.matmul(
                        pc[:, j:j + 1],
                        acc[:, l * CD + h * P: l * CD + h * P + P],
                        ones_sb[:],
                        start=True, stop=True,
                    )
            csum_sb = misc.tile([P, LP * 2], f32)
            nc.scalar.copy(csum_sb[:], pc[:])

            # ---- v rows: v[l, z] = sum_c csum[l, c] * transT[l, c, z] ------
            # PE out must start at partition 0/32/64 -> accumulate all v rows
            # as one (1, LP*ZD) PSUM row on partition 0
            pv = psum.tile([1, LP * ZD], f32, tag="ps")
            for l in range(LP):
                for h in range(2):
                    nc.tensor.matmul(
                        pv[0:1, l * ZD:(l + 1) * ZD],
                        csum_sb[:, 2 * l + h: 2 * l + h + 1],
                        tt_sb[:, l, h, :],
                        start=(h == 0), stop=(h == 1),
                    )
            v16_sb = misc.tile([1, LP * ZD], f32)
            nc.scalar.copy(v16_sb[:], pv[:])

            # ---- AllGather v (gpsimd so the sync DMA FIFO never stalls) ----
            v16_dram = dram.tile([1, LP * ZD], f32)
            vfull_dram = dram.tile([L, ZD], f32)
            nc.gpsimd.dma_start(v16_dram[:], v16_sb[:])
            nc.gpsimd.collective_compute(
                "AllGather",
                mybir.AluOpType.bypass,
                replica_groups=[list(range(NCORES))],
                ins=[v16_dram[:].opt()],
                outs=[vfull_dram[:].opt()],
            )
            v_sb = misc.tile([P, ZD], f32)
            nc.gpsimd.dma_start(v_sb[:], vfull_dram[:])

            # replicate v across the NB batch positions of a z tile
            v_rep = misc.tile([P, NB, ZD], f32)
            for b in range(NB):
                nc.vector.tensor_copy(v_rep[:, b, :], v_sb[:])

            # ---- phase Z: r^T[l, b] = sum_k z[b, l, k] v[l, k]; exp --------
            rT = misc.tile([P, BP], f32)
            z_re = z_p.rearrange("(n b) l k -> n l b k", b=NB)
            for t in range(NZT):
                zt = zpool.tile([P, NB, ZD], f32, tag="z")
                nc.sync.dma_start(zt[:], z_re[t])
                nc.vector.tensor_mul(zt[:], zt[:], v_rep[:])
                nc.vector.reduce_sum(rT[:, t * NB:(t + 1) * NB], zt[:],
                                     axis=mybir.AxisListType.X)

            out_sb = misc.tile([P, BP], f32)
            nc.scalar.activation(out_sb[:], rT[:],
                                 mybir.ActivationFunctionType.Exp)
            nc.sync.dma_start(out_p[:], out_sb[:])

    nc.compile()
    return nc


def get_program():
    global _PROGRAM
    if _PROGRAM is None:
        _PROGRAM = _build_program()
    return _PROGRAM


def shard_inputs(z, c, trans):
    z = np.ascontiguousarray(z, dtype=np.float32)
    c = np.ascontiguousarray(c, dtype=np.float32)
    trans = np.ascontiguousarray(trans, dtype=np.float32)
    in_maps = []
    for i in range(NCORES):
        ls = slice(i * LP, (i + 1) * LP)
        in_maps.append({
            "z": z[i * BP:(i + 1) * BP],
            "c": np.ascontiguousarray(c[:, ls, :]),
            "tt": np.ascontiguousarray(trans[ls].transpose(0, 2, 1)),
        })
    return in_maps


def gather_output(results):
    out = np.empty((B, L), np.float32)
    for i in range(NCORES):
        out[i * BP:(i + 1) * BP] = results[i]["out"].T
    return out


def kernel(z, c, trans):
    from concourse.bass_utils import run_bass_kernel_spmd

    nc = get_program()
    in_maps = shard_inputs(z, c, trans)
    res = run_bass_kernel_spmd(nc, in_maps, list(range(NCORES)))
    return gather_output(res.results)


# revision 8
# speedup vs baseline: 1.0630x; 1.0630x over previous
"""Trainium2 Bass kernel: density-ratio estimator loss.

Math (from the reference):
    csum = sum_b c[b, l, :]                  # (L, C)
    v[l, :] = trans[l] @ csum[l]             # (L, Z)
    r[b, l] = z[b, l, :] . v[l, :]           # (B, L)
    out = exp(r)

Sharding across 8 NeuronCores (full inputs in, full output out):
    - c     : sharded along L (16 steps per core), host-transposed to
              [l, c, b] so csum is a free-axis reduce on the Scalar (ACT)
              engine yielding column-oriented csum directly.
    - trans : sharded along L, host-pre-transposed to [l, c, z] so PE
              matmuls produce v rows (l, z) directly.
    - v     : one tiny AllGather (16x256 f32 per rank -> 128x256).
    - z     : sharded along batch (256 rows per core); out shard is
              r^T (L, B/8).

Engine split: ACT does csum reduces + final exp, PE does the v matmuls,
DVE does the z*v products, GPSIMD does the segmented k-reductions.
"""

import numpy as np

B, L, ZD, CD = 2048, 128, 256, 256
NCORES = 8
BP = B // NCORES  # 256 batches per core
LP = L // NCORES  # 16 steps per core
P = 128  # SBUF partitions

NB = 16  # batches per z tile
NZT = BP // NB  # 16 z tiles
C_BUFS = 3
Z_BUFS = 7

_PROGRAM = None


def _build_program():
    import concourse.bacc as bacc
    import concourse.mybir as mybir
    import concourse.tile as tile

    f32 = mybir.dt.float32
    nc = bacc.Bacc("TRN2", target_bir_lowering=False, debug=False,
                   num_devices=NCORES)

    z_p = nc.dram_tensor("z", [BP, L, ZD], f32, kind="ExternalInput").ap()
    ct_p = nc.dram_tensor("ct", [LP, CD, B], f32, kind="ExternalInput").ap()
    tt_p = nc.dram_tensor("tt", [LP, CD, ZD], f32, kind="ExternalInput").ap()
    out_p = nc.dram_tensor("out", [L, BP], f32, kind="ExternalOutput").ap()

    with tile.TileContext(nc) as tc:
        with (
            tc.tile_pool(name="cpool", bufs=C_BUFS) as cpool,
            tc.tile_pool(name="zpool", bufs=Z_BUFS) as zpool,
            tc.tile_pool(name="dummy", bufs=2) as dummy,
            tc.tile_pool(name="misc", bufs=1) as misc,
            tc.tile_pool(name="psum", bufs=1, space="PSUM") as psum,
            tc.tile_pool(name="dram", bufs=1, space="DRAM") as dram,
        ):
            # ---- phase C: csum columns via ACT free-axis reduction ---------
            # ct[l, c, b]; tile (128 c-half, 2048 b); accum -> csum column
            csum_sb = misc.tile([P, LP * 2], f32)
            for l in range(LP):
                for h in range(2):
                    j = 2 * l + h
                    ctile = cpool.tile([P, B], f32, tag="c")
                    nc.sync.dma_start(ctile[:], ct_p[l, h * P:(h + 1) * P, :])
                    dm = dummy.tile([P, B], f32, tag="d")
                    nc.scalar.activation(dm[:], ctile[:],
                                         mybir.ActivationFunctionType.Copy,
                                         accum_out=csum_sb[:, j:j + 1])

            # transT: whole shard resident, partitions = c (within half)
            tt_sb = misc.tile([P, LP, 2, ZD], f32)
            nc.sync.dma_start(tt_sb[:], tt_p.rearrange("l (h p) z -> p l h z", h=2))

            # ---- v rows: v[l, z] = sum_c csum[l, c] * transT[l, c, z] ------
            # PE out must start at partition 0 -> accumulate all v as one row
            pv = psum.tile([1, LP * ZD], f32, tag="ps")
            for l in range(LP):
                for h in range(2):
                    nc.tensor.matmul(
                        pv[0:1, l * ZD:(l + 1) * ZD],
                        csum_sb[:, 2 * l + h: 2 * l + h + 1],
                        tt_sb[:, l, h, :],
                        start=(h == 0), stop=(h == 1),
                    )
            v16_sb = misc.tile([1, LP * ZD], f32)
            nc.scalar.copy(v16_sb[:], pv[:])

            # ---- AllGather v (gpsimd so the sync DMA FIFO never stalls) ----
            v16_dram = dram.tile([1, LP * ZD], f32)
            vfull_dram = dram.tile([L, ZD], f32)
            nc.gpsimd.dma_start(v16_dram[:], v16_sb[:])
            nc.gpsimd.collective_compute(
                "AllGather",
                mybir.AluOpType.bypass,
                replica_groups=[list(range(NCORES))],
                ins=[v16_dram[:].opt()],
                outs=[vfull_dram[:].opt()],
            )
            v_sb = misc.tile([P, ZD], f32)
            nc.gpsimd.dma_start(v_sb[:], vfull_dram[:])
            v_bcast = v_sb[:].unsqueeze(1).broadcast_to([P, NB, ZD])

            # ---- phase Z: r^T[l, b] = sum_k z[b, l, k] v[l, k]; exp --------
            rT = misc.tile([P, BP], f32)
            z_re = z_p.rearrange("(n b) l k -> n l b k", b=NB)
            for t in range(NZT):
                zt = zpool.tile([P, NB, ZD], f32, tag="z")
                nc.sync.dma_start(zt[:], z_re[t])
                # spread the elementwise work: DVE 2/3, gpsimd 1/3
                mul_eng = nc.gpsimd if t % 3 == 2 else nc.vector
                mul_eng.tensor_mul(zt[:], zt[:], v_bcast)
                rslice = rT[:, t * NB:(t + 1) * NB]
                if t % 5 == 4:
                    # ACT-side reduce: per-b accumulate (keeps DVE free)
                    for b in range(NB):
                        adm = dummy.tile([P, ZD], f32, tag="ad")
                        nc.scalar.activation(
                            adm[:],
                            zt[:, b, :],
                            mybir.ActivationFunctionType.Copy,
                            accum_out=rslice[:, b:b + 1])
                else:
                    nc.vector.reduce_sum(rslice, zt[:],
                                         axis=mybir.AxisListType.X)

            out_sb = misc.tile([P, BP], f32)
            nc.scalar.activation(out_sb[:], rT[:],
                                 mybir.ActivationFunctionType.Exp)
            nc.sync.dma_start(out_p[:], out_sb[:])

    nc.compile()
    return nc


def get_program():
    global _PROGRAM
    if _PROGRAM is None:
        _PROGRAM = _build_program()
    return _PROGRAM


def shard_inputs(z, c, trans):
    z = np.ascontiguousarray(z, dtype=np.float32)
    c = np.ascontiguousarray(c, dtype=np.float32)
    trans = np.ascontiguousarray(trans, dtype=np.float32)
    in_maps = []
    for i in range(NCORES):
        ls = slice(i * LP, (i + 1) * LP)
        in_maps.append({
            "z": z[i * BP:(i + 1) * BP],
            "ct": np.ascontiguousarray(c[:, ls, :].transpose(1, 2, 0)),
            "tt": np.ascontiguousarray(trans[ls].transpose(0, 2, 1)),
        })
    return in_maps


def gather_output(results):
    out = np.empty((B, L), np.float32)
    for i in range(NCORES):
        out[i * BP:(i + 1) * BP] = results[i]["out"].T
    return out


def kernel(z, c, trans):
    from concourse.bass_utils import run_bass_kernel_spmd

    nc = get_program()
    in_maps = shard_inputs(z, c, trans)
    res = run_bass_kernel_spmd(nc, in_maps, list(range(NCORES)))
    return gather_output(res.results)
